# revision 33
# baseline (speedup 1.0000x reference)
"""Trainium2 Bass/Tile kernel for nn_LrFeatureUpScaler (TransformerConv on a
fully-connected graph + GraphNorm + per-node L2 norm), SPMD over 8 NeuronCores.

Sharding: target nodes (rows i) are sharded 512/core. Each core computes its
own k/v row-shard, the shards are exchanged with two AllGathers (overlapped
with the q/skip projections), attention + skip run fully local, and GraphNorm
per-channel stats are combined with one small AllReduce.

Layout strategy: all large inputs are pre-cast to bf16 and pre-transposed on
the host (x.T column-shard, x column-shard, bf16 weights), so the device loads
them straight into SBUF with no staging casts/transposes. Attention uses two
matmuls per (head, j-tile) — qk and alpha@v — with the edge-attr score term
fused into one DVE scalar_tensor_tensor, the xa*qe product on the Pool engine,
and the s = sum(alpha*xa) accumulation split between PE and DVE.

Self-contained: hardcodes all shapes; no sibling imports.
"""

import sys

for _p in ("/opt/trn_rl_repo", "/opt/trn_rl_repo/concourse"):
    if _p not in sys.path:
        sys.path.insert(0, _p)

import numpy as np
import ml_dtypes

import concourse.bass as bass
import concourse.tile as tile
from concourse import bacc, mybir
from concourse.bass_utils import run_bass_kernel_spmd
from concourse.masks import make_identity

N = 4096          # nodes == lr feature dim
H = 8             # heads
C = 512           # channels
D = C // H        # head dim = 64
M = 8             # cores
B = N // M        # rows per core = 512
PB = N // 128     # 32 p-blocks (contraction tiles)
CT = C // 128     # 4 channel tiles
JT = N // 128     # 32 j tiles
EPS = 1e-5

F32 = mybir.dt.float32
BF16 = mybir.dt.bfloat16
AF = mybir.ActivationFunctionType
ALU = mybir.AluOpType
BF16_NP = ml_dtypes.bfloat16


def _emit_once(nc, tc, io, groups, no_cc, stage=3):
    (xT_in, xa_in, Wq, Wk, Wv, Wskip, bq, bk, bv, bskip, we,
     gn_w, gn_b, gn_ms, out) = io
    with (
        tc.tile_pool(name="consts", bufs=1) as consts,
        tc.tile_pool(name="keep", bufs=1) as keep,
        tc.tile_pool(name="small", bufs=2) as small,
        tc.tile_pool(name="dram", bufs=1, space="DRAM") as dram,
    ):
        # ---------------- constants ----------------
        ident_f32 = consts.tile([128, 128], F32)
        make_identity(nc, ident_f32)

        # per-c-tile vectors: [128, CT] layout c = ct*128 + p
        def load_cvec(dram_t, dt=F32):
            t = consts.tile([128, CT], dt, name=f"cvec_{dram_t.name}")
            nc.gpsimd.dma_start(
                out=t, in_=dram_t.ap().rearrange("(t p) -> p t", p=128)
            )
            return t

        bq_sb = load_cvec(bq)
        bk_sb = load_cvec(bk)
        bskip_sb = load_cvec(bskip)
        gnw_sb = load_cvec(gn_w)
        gnb_sb = load_cvec(gn_b)
        gnms_sb = load_cvec(gn_ms)
        we_col = load_cvec(we)

        # bv broadcast across partitions: [128, C] f32
        bv_b = consts.tile([128, C], F32)
        nc.gpsimd.dma_start(
            out=bv_b, in_=bv.ap().unsqueeze(0).partition_broadcast(128)
        )

        # we_aug [128, H, D+1] (col D stays 0): bf16 for per-tile PE s-folds,
        # f32 for the end-of-head fold of the DVE accumulator.
        we_aug_bf = consts.tile([128, H, D + 1], BF16)
        nc.vector.memset(we_aug_bf, 0.0)
        for h in range(H):
            nc.gpsimd.dma_start(
                out=we_aug_bf[:, h, 0:D],
                in_=we.ap()[h * D:(h + 1) * D].unsqueeze(0).partition_broadcast(128),
            )

        eps_col = consts.tile([128, 1], F32)
        nc.vector.memset(eps_col, EPS)
        ones_row = consts.tile([1, 128], F32)
        nc.vector.memset(ones_row, 1.0)
        # eye8x64[k, h, d] = 1 if k == h: broadcasts row h of an [8, B] tile
        # onto 64 partitions via one matmul. DVE cannot write partition bases
        # 1..7, so the one-rows are placed with SBUF->SBUF DMAs.
        eye8x64 = consts.tile([8, H, D], F32)
        nc.vector.memset(eye8x64, 0.0)
        one64 = consts.tile([1, D], F32)
        nc.vector.memset(one64, 1.0)
        for h in range(H):
            nc.gpsimd.dma_start(out=eye8x64[h:h + 1, h, :], in_=one64)
        # we_rep8[cp0+d, h, m] = 0.125*we[h*D+d] for all m — stationary operand
        # that computes qe pre-scaled by 1/sqrt(D) and broadcast across
        # partitions.
        ones8_blk = consts.tile([128, 128], BF16)
        nc.vector.memset(ones8_blk, 1.0)
        we_rep8 = consts.tile([128, H, 128], BF16)
        for h in range(H):
            cp0 = (h % 2) * D
            nc.vector.tensor_scalar(
                out=we_rep8[cp0:cp0 + D, h, :],
                in0=ones8_blk[cp0:cp0 + D, :],
                scalar1=we_col[cp0:cp0 + D, h // 2:h // 2 + 1],
                scalar2=None,
                op0=ALU.mult,
            )

        # ---------------- bulk input loads (no staging) ----------------
        cc_in_k = dram.tile([B, C], BF16)
        cc_in_v = dram.tile([B, C], BF16)
        cc_out_k = dram.tile([M, B, C], BF16, addr_space="Shared")
        cc_out_v = dram.tile([M, B, C], BF16, addr_space="Shared")

        kT = keep.tile([128, CT, N], BF16)     # kT[cp, ct, j]
        qT = keep.tile([128, CT, B], BF16)     # qT[cp, ct, i]
        outT = keep.tile([128, CT, B], F32)    # pre-norm out, transposed
        xa = keep.tile([128, JT, B], BF16)     # xa[jp, jt, i] = x[j_glob, i_glob]
        v_aug = keep.tile([128, JT, H, D + 1], BF16)
        nc.vector.memset(v_aug[:, :, :, D:D + 1], 1.0)
        qe8_b = keep.tile([128, H, B], BF16)

        with (
            tc.tile_pool(name="xtpool", bufs=1) as xtpool,
            tc.tile_pool(name="wstream", bufs=1) as wstream,
            tc.tile_pool(name="locstage", bufs=1) as locstage,
            tc.tile_pool(name="psum_proj", bufs=1, space="PSUM") as pp,
        ):
            # xT [128, PB, B] bf16: xT[p, pb, i] = x[i_global, pb*128+p]
            # batched 4-tile DMAs to cut queue trigger time
            xT = xtpool.tile([128, PB, B], BF16)
            for q4 in range(PB // 4):
                nc.sync.dma_start(
                    out=xT[:, q4 * 4:(q4 + 1) * 4, :],
                    in_=xT_in[q4 * 512:(q4 + 1) * 512, :].rearrange(
                        "(q p) i -> p q i", p=128
                    ),
                )
            # xa loads (needed only at attention time) follow the xT loads
            for q4 in range(JT // 4):
                nc.sync.dma_start(
                    out=xa[:, q4 * 4:(q4 + 1) * 4, :],
                    in_=xa_in[q4 * 512:(q4 + 1) * 512, :].rearrange(
                        "(q p) i -> p q i", p=128
                    ),
                )

            def projT(W_dram, bias_sb, dst_ap, scale=None):
                """dst[cp, ct, i] = (sum_p xT[p,:,i]*W[p,ct*128+cp] + b)*scale."""
                psums = [
                    pp.tile([128, B], F32, tag=f"pp{ct}",
                            name=f"psum_{W_dram.name}_{ct}")
                    for ct in range(CT)
                ]
                for q4 in range(PB // 4):
                    wt4 = wstream.tile([128, 4, C], BF16, tag="wt", bufs=2,
                                       name=f"wt_{W_dram.name}")
                    nc.scalar.dma_start(
                        out=wt4,
                        in_=W_dram[q4 * 512:(q4 + 1) * 512, :].rearrange(
                            "(q p) c -> p q c", p=128
                        ),
                    )
                    for q in range(4):
                        pb = q4 * 4 + q
                        for ct in range(CT):
                            nc.tensor.matmul(
                                psums[ct],
                                lhsT=wt4[:, q, ct * 128:(ct + 1) * 128],
                                rhs=xT[:, pb, :],
                                start=(pb == 0),
                                stop=(pb == PB - 1),
                            )
                for ct in range(CT):
                    if scale is None:
                        nc.vector.tensor_scalar(
                            out=dst_ap[:, ct, :],
                            in0=psums[ct],
                            scalar1=bias_sb[:, ct:ct + 1],
                            scalar2=None,
                            op0=ALU.add,
                        )
                    else:
                        nc.vector.tensor_scalar(
                            out=dst_ap[:, ct, :],
                            in0=psums[ct],
                            scalar1=bias_sb[:, ct:ct + 1],
                            scalar2=scale,
                            op0=ALU.add,
                            op1=ALU.mult,
                        )

            # -------- k shard + AllGather (fired early) --------
            kT_loc = locstage.tile([128, CT, B], BF16)
            projT(Wk, bk_sb, kT_loc)
            nc.sync.dma_start(
                out=cc_in_k.rearrange("(ct cp) j -> cp ct j", cp=128),
                in_=kT_loc,
            )
            if no_cc:
                for r in range(M):
                    nc.sync.dma_start(out=cc_out_k[r], in_=cc_in_k)
            else:
                nc.gpsimd.collective_compute(
                    "AllGather",
                    ALU.bypass,
                    replica_groups=groups,
                    ins=[cc_in_k.opt()],
                    outs=[cc_out_k.opt()],
                )

            # -------- v shard + AllGather (pow2 payload) --------
            # v[jp, jtl, c] = sum_p x[jtl*128+jp, p]*Wv[p, c]; Wv read once.
            v_loc = locstage.tile([128, 4, H, D], BF16)
            psvs = [
                pp.tile([128, C], F32, tag=f"pp{jtl}", name=f"psum_v_{jtl}")
                for jtl in range(4)
            ]
            for q4 in range(PB // 4):
                wt4 = wstream.tile([128, 4, C], BF16, tag="wt", bufs=2,
                                   name="wt_v")
                nc.scalar.dma_start(
                    out=wt4,
                    in_=Wv[q4 * 512:(q4 + 1) * 512, :].rearrange(
                        "(q p) c -> p q c", p=128
                    ),
                )
                for q in range(4):
                    pb = q4 * 4 + q
                    for jtl in range(4):
                        nc.tensor.matmul(
                            psvs[jtl],
                            lhsT=xT[:, pb, jtl * 128:(jtl + 1) * 128],
                            rhs=wt4[:, q, :],
                            start=(pb == 0),
                            stop=(pb == PB - 1),
                        )
            for jtl in range(4):
                nc.vector.tensor_tensor(
                    out=v_loc[:, jtl, :, :],
                    in0=psvs[jtl].rearrange("p (h d) -> p h d", h=H),
                    in1=bv_b.rearrange("p (h d) -> p h d", h=H),
                    op=ALU.add,
                )
            nc.sync.dma_start(
                out=cc_in_v.rearrange("(jtl jp) f -> jp jtl f", jp=128),
                in_=v_loc,
            )
            if no_cc:
                for r in range(M):
                    nc.sync.dma_start(out=cc_out_v[r], in_=cc_in_v)
            else:
                nc.gpsimd.collective_compute(
                    "AllGather",
                    ALU.bypass,
                    replica_groups=groups,
                    ins=[cc_in_v.opt()],
                    outs=[cc_out_v.opt()],
                )

            # -------- q, skip, qe (overlap the AllGathers) --------
            projT(Wq, bq_sb, qT, scale=0.125)
            projT(Wskip, bskip_sb, outT)

            # qe8_b[:, h, i] = 0.125 * sum_d we[h*D+d]*qT[h*D+d, i]
            for h in range(H):
                cp0 = (h % 2) * D
                ct = h // 2
                pq = pp.tile([128, B], F32, tag="pq", bufs=2, name=f"psum_qe_{h}")
                nc.tensor.matmul(
                    pq,
                    lhsT=we_rep8[cp0:cp0 + D, h, :],
                    rhs=qT[cp0:cp0 + D, ct, :],
                )
                nc.vector.tensor_copy(out=qe8_b[:, h, :], in_=pq)

        if stage <= 1:
            for ct in range(CT):
                nc.sync.dma_start(
                    out=out[ct * 128:(ct + 1) * 128, :], in_=outT[:, ct, :]
                )
            return

        # ---------------- unpack gathered kT / v (contiguous HWDGE loads) ----
        for r in range(M):
            nc.sync.dma_start(
                out=kT[:, :, r * B:(r + 1) * B],
                in_=cc_out_k[r].rearrange("(ct cp) j -> cp ct j", cp=128),
            )
            for jtl in range(4):
                nc.scalar.dma_start(
                    out=v_aug[:, r * 4 + jtl, :, 0:D],
                    in_=cc_out_v[r, jtl * 128:(jtl + 1) * 128, :].rearrange(
                        "p (h d) -> p h d", h=H
                    ),
                )

        stats = small.tile([128, 2 * CT], F32, bufs=1)
        zall = keep.tile([H, B], F32)
        po_num = keep.tile([D, H, B], F32)

        # ---------------- attention ----------------
        # Grouped inner loop: qk matmuls for group g+1 are emitted before the
        # alpha@v matmuls of group g, so the PE queue streams back-to-back
        # matmuls (stays in its fast p-state) instead of gapping on the
        # vector->scalar->vector alpha chain. qT is pre-scaled by 0.125, so
        # the score add is a plain tensor add.
        GROUP = 4
        with (
            tc.tile_pool(name="psum_att", bufs=1, space="PSUM") as pa,
            tc.tile_pool(name="att", bufs=1) as att,
        ):
            for h in range(H):
                cp0 = (h % 2) * D
                ct = h // 2
                total_po = 2 * JT
                # double-banked accumulator: strict alternation avoids
                # back-to-back PSUM read-modify-write stalls in the dense
                # matmul streaks
                po = [
                    pa.tile([D + 1, B], F32, tag=f"po{b}", bufs=1,
                            name=f"po{b}_{h}")
                    for b in range(2)
                ]
                po_k = 0

                def po_mm(lhsT, rhs):
                    nonlocal po_k
                    nc.tensor.matmul(
                        po[po_k % 2],
                        lhsT=lhsT,
                        rhs=rhs,
                        start=(po_k < 2),
                        stop=(po_k >= total_po - 2),
                        skip_group_check=True,
                    )
                    po_k += 1

                jt_groups = [
                    list(range(g, min(g + GROUP, JT)))
                    for g in range(0, JT, GROUP)
                ]
                ps_of = {}

                def emit_qk(jts):
                    for jt in jts:
                        ps = pa.tile([128, B], F32, tag="ps", bufs=5,
                                     name=f"ps_{h}_{jt}")
                        ps_of[jt] = ps
                        nc.tensor.matmul(
                            ps,
                            lhsT=kT[cp0:cp0 + D, ct, jt * 128:(jt + 1) * 128],
                            rhs=qT[cp0:cp0 + D, ct, :],
                            start=True,
                            stop=True,
                        )

                def emit_body(jts):
                    tiles = {}
                    for jt in jts:
                        tmp = att.tile([128, B], BF16, tag="tmp", bufs=6,
                                       name=f"tmp_{h}_{jt}")
                        nc.vector.tensor_tensor(
                            out=tmp, in0=xa[:, jt, :], in1=qe8_b[:, h, :],
                            op=ALU.mult,
                        )
                        ps2 = att.tile([128, B], BF16, tag="ps2", bufs=6,
                                       name=f"ps2_{h}_{jt}")
                        nc.vector.tensor_tensor(
                            out=ps2, in0=ps_of.pop(jt), in1=tmp, op=ALU.add
                        )
                        alpha = att.tile([128, B], BF16, tag="alpha", bufs=6,
                                         name=f"alpha_{h}_{jt}")
                        nc.scalar.activation(
                            out=alpha, in_=ps2, func=AF.Exp, scale=1.0
                        )
                        tiles[jt] = alpha
                    for jt in jts:
                        alpha = tiles[jt]
                        po_mm(v_aug[:, jt, h, :], alpha)
                        mt = att.tile([128, B], BF16, tag="mt", bufs=6,
                                      name=f"mt_{h}_{jt}")
                        nc.vector.tensor_tensor(
                            out=mt, in0=alpha, in1=xa[:, jt, :], op=ALU.mult
                        )
                        po_mm(we_aug_bf[:, h, :], mt)

                for g, jts in enumerate(jt_groups):
                    emit_qk(jts)
                    if g > 0:
                        emit_body(jt_groups[g - 1])
                emit_body(jt_groups[-1])
                assert po_k == total_po
                # epilogue part 1: merge the two banks, stash numerator and
                # Z row (zall[h]; partition h is DMA-reachable only)
                ztmp = small.tile([1, B], F32, tag="ztmp", name=f"ztmp_{h}")
                nc.vector.tensor_copy(out=ztmp, in_=po[0][D:D + 1, :])
                nc.vector.tensor_tensor(
                    out=ztmp, in0=ztmp, in1=po[1][D:D + 1, :], op=ALU.add
                )
                nc.sync.dma_start(out=zall[h:h + 1, :], in_=ztmp)
                nc.vector.tensor_copy(out=po_num[:, h, :], in_=po[0][0:D, :])
                nc.vector.tensor_tensor(
                    out=po_num[:, h, :], in0=po_num[:, h, :],
                    in1=po[1][0:D, :], op=ALU.add,
                )

            # epilogue part 2: one batched reciprocal, then per-head
            # broadcast + multiply-accumulate into outT
            rzall = att.tile([H, B], F32)
            nc.vector.reciprocal(out=rzall, in_=zall)
            for h in range(H):
                cp0 = (h % 2) * D
                ct = h // 2
                prz = pa.tile([D, B], F32, tag="prz", bufs=1, name=f"prz2_{h}")
                nc.tensor.matmul(prz, lhsT=eye8x64[:, h, :], rhs=rzall)
                t1f = small.tile([128, B], F32, tag="t1", name=f"t1_{h}")
                t1 = t1f[cp0:cp0 + D, :]
                nc.vector.tensor_tensor(
                    out=t1, in0=po_num[:, h, :], in1=prz, op=ALU.mult
                )
                nc.vector.tensor_tensor(
                    out=outT[cp0:cp0 + D, ct, :],
                    in0=outT[cp0:cp0 + D, ct, :],
                    in1=t1,
                    op=ALU.add,
                )
                if h % 2 == 1:
                    # outT[:, ct, :] is final: fold its GraphNorm stats so the
                    # AllReduce fires right after the last head
                    sm = small.tile([128, 1], F32, tag="sm", name=f"sm_{ct}")
                    nc.vector.tensor_reduce(
                        out=sm, in_=outT[:, ct, :], axis=mybir.AxisListType.X,
                        op=ALU.add,
                    )
                    nc.vector.tensor_scalar(
                        out=stats[:, 2 * ct:2 * ct + 1], in0=sm,
                        scalar1=1.0 / N, scalar2=None, op0=ALU.mult,
                    )
                    scr = att.tile([128, B], F32, tag="scr", bufs=2,
                                   name=f"scr_{ct}")
                    nc.scalar.activation(
                        out=scr, in_=outT[:, ct, :], func=AF.Square
                    )
                    ss = small.tile([128, 1], F32, tag="ss", name=f"ss_{ct}")
                    nc.vector.tensor_reduce(
                        out=ss, in_=scr, axis=mybir.AxisListType.X, op=ALU.add
                    )
                    nc.vector.tensor_scalar(
                        out=stats[:, 2 * ct + 1:2 * ct + 2], in0=ss,
                        scalar1=1.0 / N, scalar2=None, op0=ALU.mult,
                    )

        if stage <= 2:
            for ct in range(CT):
                nc.sync.dma_start(
                    out=out[ct * 128:(ct + 1) * 128, :], in_=outT[:, ct, :]
                )
            return

        # ---------------- GraphNorm + L2 + emit ----------------
        with (
            tc.tile_pool(name="fin", bufs=1) as fin,
            tc.tile_pool(name="psum_f", bufs=1, space="PSUM") as pf,
        ):
            st_in = dram.tile([128, 2 * CT], F32)
            st_out = dram.tile([128, 2 * CT], F32, addr_space="Shared")
            nc.sync.dma_start(out=st_in, in_=stats)
            if no_cc:
                nc.sync.dma_start(out=st_out, in_=st_in)
            else:
                nc.gpsimd.collective_compute(
                    "AllReduce",
                    ALU.add,
                    replica_groups=groups,
                    ins=[st_in.opt()],
                    outs=[st_out.opt()],
                )

            # transpose raw outT while the AllReduce runs
            pre = fin.tile([128, 4, C], F32)  # [ip, it, c]
            for ct in range(CT):
                for it in range(4):
                    pt = pf.tile([128, 128], F32, tag="pt", bufs=4,
                                 name=f"pt_{ct}_{it}")
                    nc.tensor.transpose(
                        pt, outT[:, ct, it * 128:(it + 1) * 128], ident_f32
                    )
                    nc.vector.tensor_copy(
                        out=pre[:, it, ct * 128:(ct + 1) * 128], in_=pt
                    )

            gstats = small.tile([128, 2 * CT], F32, bufs=1)
            nc.sync.dma_start(out=gstats, in_=st_out)

            # per-channel A (scale) and Bc (shift), in one cvec tile so the
            # broadcast round-trip is a single DMA pair
            AB_cvec = small.tile([128, 2 * CT], F32, bufs=1)
            A_cvec = AB_cvec[:, 0:CT]
            B_cvec = AB_cvec[:, CT:2 * CT]
            for ct in range(CT):
                EX = gstats[:, 2 * ct:2 * ct + 1]
                EX2 = gstats[:, 2 * ct + 1:2 * ct + 2]
                msv = gnms_sb[:, ct:ct + 1]
                t2 = small.tile([128, 1], F32, tag="n_t", name=f"nt_{ct}")
                nc.vector.tensor_tensor(out=t2, in0=EX, in1=EX, op=ALU.mult)
                w1 = small.tile([128, 1], F32, tag="n_w", name=f"nw_{ct}")
                nc.vector.tensor_scalar(
                    out=w1, in0=msv, scalar1=-1.0, scalar2=2.0,
                    op0=ALU.mult, op1=ALU.add,
                )  # 2 - ms
                nc.vector.tensor_tensor(out=w1, in0=msv, in1=w1, op=ALU.mult)
                nc.vector.tensor_tensor(out=t2, in0=t2, in1=w1, op=ALU.mult)
                var = small.tile([128, 1], F32, tag="n_var", name=f"nvar_{ct}")
                nc.vector.tensor_tensor(out=var, in0=EX2, in1=t2, op=ALU.subtract)
                sd = small.tile([128, 1], F32, tag="n_sd", name=f"nsd_{ct}")
                nc.scalar.activation(out=sd, in_=var, func=AF.Sqrt, bias=eps_col)
                rstd = small.tile([128, 1], F32, tag="n_rstd", name=f"nrstd_{ct}")
                nc.vector.reciprocal(out=rstd, in_=sd)
                nc.vector.tensor_tensor(
                    out=A_cvec[:, ct:ct + 1], in0=gnw_sb[:, ct:ct + 1],
                    in1=rstd, op=ALU.mult,
                )
                p1 = small.tile([128, 1], F32, tag="n_p1", name=f"np1_{ct}")
                nc.vector.tensor_tensor(
                    out=p1, in0=A_cvec[:, ct:ct + 1], in1=msv, op=ALU.mult
                )
                nc.vector.tensor_tensor(out=p1, in0=p1, in1=EX, op=ALU.mult)
                nc.vector.tensor_tensor(
                    out=B_cvec[:, ct:ct + 1], in0=gnb_sb[:, ct:ct + 1],
                    in1=p1, op=ALU.subtract,
                )

            # broadcast A/Bc along partitions via one DRAM round trip (tiny)
            AB_dram = dram.tile([2 * C], F32)
            nc.gpsimd.dma_start(
                out=AB_dram.rearrange("(t p) -> p t", p=128), in_=AB_cvec
            )
            AB_bcast = fin.tile([128, 2 * C], F32)
            nc.gpsimd.dma_start(
                out=AB_bcast,
                in_=AB_dram.unsqueeze(0).partition_broadcast(128),
            )
            A_bcast = AB_bcast.rearrange("p (two c) -> p two c", two=2)[:, 0, :]
            B_bcast = AB_bcast.rearrange("p (two c) -> p two c", two=2)[:, 1, :]

            final = fin.tile([128, 4, C], F32)
            sqj = fin.tile([128, C], F32)
            for it in range(4):
                nc.vector.tensor_tensor(
                    out=final[:, it, :], in0=pre[:, it, :], in1=A_bcast,
                    op=ALU.mult,
                )
                nc.vector.tensor_tensor(
                    out=final[:, it, :], in0=final[:, it, :], in1=B_bcast,
                    op=ALU.add,
                )
                l2 = small.tile([128, 1], F32, tag="l2", name=f"l2_{it}")
                nc.scalar.activation(out=sqj, in_=final[:, it, :], func=AF.Square)
                nc.vector.tensor_reduce(
                    out=l2, in_=sqj, axis=mybir.AxisListType.X, op=ALU.add
                )
                sd2 = small.tile([128, 1], F32, tag="sd2", name=f"sd2_{it}")
                nc.scalar.activation(out=sd2, in_=l2, func=AF.Sqrt)
                rn = small.tile([128, 1], F32, tag="rn", name=f"rn_{it}")
                nc.vector.reciprocal(out=rn, in_=sd2)
                nc.vector.tensor_scalar(
                    out=final[:, it, :], in0=final[:, it, :],
                    scalar1=rn, scalar2=None, op0=ALU.mult,
                )
                nc.sync.dma_start(
                    out=out[it * 128:(it + 1) * 128, :], in_=final[:, it, :]
                )


def build_kernel(no_cc=False, n_cores=M, repeat=1, stage=3):
    nc = bacc.Bacc("TRN2", target_bir_lowering=False, debug=False,
                   num_devices=n_cores)

    xT_in = nc.dram_tensor("xT_bf", [N, B], BF16, kind="ExternalInput")
    xa_in = nc.dram_tensor("xa_bf", [N, B], BF16, kind="ExternalInput")
    Wq = nc.dram_tensor("Wq_bf", [N, C], BF16, kind="ExternalInput")
    Wk = nc.dram_tensor("Wk_bf", [N, C], BF16, kind="ExternalInput")
    Wv = nc.dram_tensor("Wv_bf", [N, C], BF16, kind="ExternalInput")
    Wskip = nc.dram_tensor("Wskip_bf", [N, C], BF16, kind="ExternalInput")
    bq = nc.dram_tensor("bq", [C], F32, kind="ExternalInput")
    bk = nc.dram_tensor("bk", [C], F32, kind="ExternalInput")
    bv = nc.dram_tensor("bv", [C], F32, kind="ExternalInput")
    bskip = nc.dram_tensor("bskip", [C], F32, kind="ExternalInput")
    we = nc.dram_tensor("we", [C], F32, kind="ExternalInput")
    gn_w = nc.dram_tensor("gn_w", [C], F32, kind="ExternalInput")
    gn_b = nc.dram_tensor("gn_b", [C], F32, kind="ExternalInput")
    gn_ms = nc.dram_tensor("gn_ms", [C], F32, kind="ExternalInput")
    out = nc.dram_tensor("out", [B, C], F32, kind="ExternalOutput")

    io = (xT_in, xa_in, Wq, Wk, Wv, Wskip, bq, bk, bv, bskip, we,
          gn_w, gn_b, gn_ms, out)
    groups = [list(range(n_cores))]

    with tile.TileContext(nc) as tc:
        for _rep in range(repeat):
            _emit_once(nc, tc, io, groups, no_cc, stage=stage)

    nc.finalize()
    return nc


_NC_CACHE = {}


def make_in_maps(inputs):
    """Host-side prep: slice per-core shards, pre-transpose x, cast to bf16."""
    x = np.ascontiguousarray(inputs["x"], dtype=np.float32)
    xT_bf = np.ascontiguousarray(x.T).astype(BF16_NP)   # [N, N]; col i = x[i,:]
    x_bf = x.astype(BF16_NP)
    w_bf = {
        k: np.ascontiguousarray(inputs[k], dtype=np.float32).astype(BF16_NP)
        for k in ("Wq", "Wk", "Wv", "Wskip")
    }
    f32v = {
        k: np.ascontiguousarray(inputs[k], dtype=np.float32)
        for k in ("bq", "bk", "bv", "bskip", "we", "gn_w", "gn_b", "gn_ms")
    }
    in_maps = []
    for m in range(M):
        I = slice(m * B, (m + 1) * B)
        im = {
            "xT_bf": np.ascontiguousarray(xT_bf[:, I]),
            "xa_bf": np.ascontiguousarray(x_bf[:, I]),
            "Wq_bf": w_bf["Wq"], "Wk_bf": w_bf["Wk"],
            "Wv_bf": w_bf["Wv"], "Wskip_bf": w_bf["Wskip"],
        }
        im.update(f32v)
        in_maps.append(im)
    return in_maps


def kernel(**inputs):
    if "nc" not in _NC_CACHE:
        _NC_CACHE["nc"] = build_kernel()
    nc = _NC_CACHE["nc"]
    in_maps = make_in_maps(inputs)
    res = run_bass_kernel_spmd(nc, in_maps, core_ids=list(range(M)))
    return np.concatenate([res.results[m]["out"] for m in range(M)], axis=0)


if __name__ == "__main__":
    data = np.load("/tmp/inputs.npz")
    out = kernel(**{k: data[k] for k in data.files})
    ref = np.load("/tmp/ref_out.npy")
    err = np.abs(out - ref)
    print("absmax", err.max(), "scale-rel", err.max() / np.abs(ref).max())
    print("rel2", np.linalg.norm(out - ref) / np.linalg.norm(ref))


# revision 34
# speedup vs baseline: 1.0080x; 1.0080x over previous
"""Trainium2 Bass/Tile kernel for nn_LrFeatureUpScaler (TransformerConv on a
fully-connected graph + GraphNorm + per-node L2 norm), SPMD over 8 NeuronCores.

Sharding: target nodes (rows i) are sharded 512/core. Each core computes its
own k/v row-shard, the shards are exchanged with two AllGathers (overlapped
with the q/skip projections), attention + skip run fully local, and GraphNorm
per-channel stats are combined with one small AllReduce.

Layout strategy: all large inputs are pre-cast to bf16 and pre-transposed on
the host (x.T column-shard, x column-shard, bf16 weights), so the device loads
them straight into SBUF with no staging casts/transposes. Attention uses two
matmuls per (head, j-tile) — qk and alpha@v — with the edge-attr score term
fused into one DVE scalar_tensor_tensor, the xa*qe product on the Pool engine,
and the s = sum(alpha*xa) accumulation split between PE and DVE.

Self-contained: hardcodes all shapes; no sibling imports.
"""

import sys

for _p in ("/opt/trn_rl_repo", "/opt/trn_rl_repo/concourse"):
    if _p not in sys.path:
        sys.path.insert(0, _p)

import numpy as np
import ml_dtypes

import concourse.bass as bass
import concourse.tile as tile
from concourse import bacc, mybir
from concourse.bass_utils import run_bass_kernel_spmd
from concourse.masks import make_identity

N = 4096          # nodes == lr feature dim
H = 8             # heads
C = 512           # channels
D = C // H        # head dim = 64
M = 8             # cores
B = N // M        # rows per core = 512
PB = N // 128     # 32 p-blocks (contraction tiles)
CT = C // 128     # 4 channel tiles
JT = N // 128     # 32 j tiles
EPS = 1e-5

F32 = mybir.dt.float32
BF16 = mybir.dt.bfloat16
AF = mybir.ActivationFunctionType
ALU = mybir.AluOpType
BF16_NP = ml_dtypes.bfloat16


def _emit_once(nc, tc, io, groups, no_cc, stage=3):
    (xT_in, xa_in, Wq, Wk, Wv, Wskip, bq, bk, bv, bskip, we,
     gn_w, gn_b, gn_ms, out) = io
    with (
        tc.tile_pool(name="consts", bufs=1) as consts,
        tc.tile_pool(name="keep", bufs=1) as keep,
        tc.tile_pool(name="small", bufs=2) as small,
        tc.tile_pool(name="dram", bufs=1, space="DRAM") as dram,
    ):
        # ---------------- constants ----------------
        ident_f32 = consts.tile([128, 128], F32)
        make_identity(nc, ident_f32)

        # per-c-tile vectors: [128, CT] layout c = ct*128 + p
        def load_cvec(dram_t, dt=F32):
            t = consts.tile([128, CT], dt, name=f"cvec_{dram_t.name}")
            nc.gpsimd.dma_start(
                out=t, in_=dram_t.ap().rearrange("(t p) -> p t", p=128)
            )
            return t

        bq_sb = load_cvec(bq)
        bk_sb = load_cvec(bk)
        bskip_sb = load_cvec(bskip)
        gnw_sb = load_cvec(gn_w)
        gnb_sb = load_cvec(gn_b)
        gnms_sb = load_cvec(gn_ms)
        we_col = load_cvec(we)

        # bv broadcast across partitions: [128, C] f32
        bv_b = consts.tile([128, C], F32)
        nc.gpsimd.dma_start(
            out=bv_b, in_=bv.ap().unsqueeze(0).partition_broadcast(128)
        )

        # we_aug [128, H, D+1] (col D stays 0): bf16 for per-tile PE s-folds,
        # f32 for the end-of-head fold of the DVE accumulator.
        we_aug_bf = consts.tile([128, H, D + 1], BF16)
        nc.vector.memset(we_aug_bf, 0.0)
        for h in range(H):
            nc.gpsimd.dma_start(
                out=we_aug_bf[:, h, 0:D],
                in_=we.ap()[h * D:(h + 1) * D].unsqueeze(0).partition_broadcast(128),
            )

        eps_col = consts.tile([128, 1], F32)
        nc.vector.memset(eps_col, EPS)
        ones_row = consts.tile([1, 128], F32)
        nc.vector.memset(ones_row, 1.0)
        # eye8x64[k, h, d] = 1 if k == h: broadcasts row h of an [8, B] tile
        # onto 64 partitions via one matmul. DVE cannot write partition bases
        # 1..7, so the one-rows are placed with SBUF->SBUF DMAs.
        eye8x64 = consts.tile([8, H, D], F32)
        nc.vector.memset(eye8x64, 0.0)
        one64 = consts.tile([1, D], F32)
        nc.vector.memset(one64, 1.0)
        for h in range(H):
            nc.gpsimd.dma_start(out=eye8x64[h:h + 1, h, :], in_=one64)
        # we_rep8[cp0+d, h, m] = 0.125*we[h*D+d] for all m — stationary operand
        # that computes qe pre-scaled by 1/sqrt(D) and broadcast across
        # partitions.
        ones8_blk = consts.tile([128, 128], BF16)
        nc.vector.memset(ones8_blk, 1.0)
        we_rep8 = consts.tile([128, H, 128], BF16)
        for h in range(H):
            cp0 = (h % 2) * D
            nc.vector.tensor_scalar(
                out=we_rep8[cp0:cp0 + D, h, :],
                in0=ones8_blk[cp0:cp0 + D, :],
                scalar1=we_col[cp0:cp0 + D, h // 2:h // 2 + 1],
                scalar2=None,
                op0=ALU.mult,
            )

        # ---------------- bulk input loads (no staging) ----------------
        cc_in_k = dram.tile([B, C], BF16)
        cc_in_v = dram.tile([B, C], BF16)
        cc_out_k = dram.tile([M, B, C], BF16, addr_space="Shared")
        cc_out_v = dram.tile([M, B, C], BF16, addr_space="Shared")

        kT = keep.tile([128, CT, N], BF16)     # kT[cp, ct, j]
        qT = keep.tile([128, CT, B], BF16)     # qT[cp, ct, i]
        outT = keep.tile([128, CT, B], F32)    # pre-norm out, transposed
        xa = keep.tile([128, JT, B], BF16)     # xa[jp, jt, i] = x[j_glob, i_glob]
        v_aug = keep.tile([128, JT, H, D + 1], BF16)
        nc.vector.memset(v_aug[:, :, :, D:D + 1], 1.0)
        qe8_b = keep.tile([128, H, B], BF16)

        with (
            tc.tile_pool(name="xtpool", bufs=1) as xtpool,
            tc.tile_pool(name="wstream", bufs=1) as wstream,
            tc.tile_pool(name="locstage", bufs=1) as locstage,
            tc.tile_pool(name="psum_proj", bufs=1, space="PSUM") as pp,
        ):
            # xT [128, PB, B] bf16: xT[p, pb, i] = x[i_global, pb*128+p]
            # batched 4-tile DMAs to cut queue trigger time
            xT = xtpool.tile([128, PB, B], BF16)
            for q4 in range(PB // 4):
                nc.sync.dma_start(
                    out=xT[:, q4 * 4:(q4 + 1) * 4, :],
                    in_=xT_in[q4 * 512:(q4 + 1) * 512, :].rearrange(
                        "(q p) i -> p q i", p=128
                    ),
                )
            # xa loads (needed only at attention time) follow the xT loads
            for q4 in range(JT // 4):
                nc.sync.dma_start(
                    out=xa[:, q4 * 4:(q4 + 1) * 4, :],
                    in_=xa_in[q4 * 512:(q4 + 1) * 512, :].rearrange(
                        "(q p) i -> p q i", p=128
                    ),
                )

            def projT(W_dram, bias_sb, dst_ap, scale=None):
                """dst[cp, ct, i] = (sum_p xT[p,:,i]*W[p,ct*128+cp] + b)*scale."""
                psums = [
                    pp.tile([128, B], F32, tag=f"pp{ct}",
                            name=f"psum_{W_dram.name}_{ct}")
                    for ct in range(CT)
                ]
                for q4 in range(PB // 4):
                    wt4 = wstream.tile([128, 4, C], BF16, tag="wt", bufs=2,
                                       name=f"wt_{W_dram.name}")
                    nc.scalar.dma_start(
                        out=wt4,
                        in_=W_dram[q4 * 512:(q4 + 1) * 512, :].rearrange(
                            "(q p) c -> p q c", p=128
                        ),
                    )
                    for q in range(4):
                        pb = q4 * 4 + q
                        for ct in range(CT):
                            nc.tensor.matmul(
                                psums[ct],
                                lhsT=wt4[:, q, ct * 128:(ct + 1) * 128],
                                rhs=xT[:, pb, :],
                                start=(pb == 0),
                                stop=(pb == PB - 1),
                            )
                for ct in range(CT):
                    if scale is None:
                        nc.vector.tensor_scalar(
                            out=dst_ap[:, ct, :],
                            in0=psums[ct],
                            scalar1=bias_sb[:, ct:ct + 1],
                            scalar2=None,
                            op0=ALU.add,
                        )
                    else:
                        nc.vector.tensor_scalar(
                            out=dst_ap[:, ct, :],
                            in0=psums[ct],
                            scalar1=bias_sb[:, ct:ct + 1],
                            scalar2=scale,
                            op0=ALU.add,
                            op1=ALU.mult,
                        )

            # -------- k shard + AllGather (fired early) --------
            kT_loc = locstage.tile([128, CT, B], BF16)
            projT(Wk, bk_sb, kT_loc)
            nc.sync.dma_start(
                out=cc_in_k.rearrange("(ct cp) j -> cp ct j", cp=128),
                in_=kT_loc,
            )
            if no_cc:
                for r in range(M):
                    nc.sync.dma_start(out=cc_out_k[r], in_=cc_in_k)
            else:
                nc.gpsimd.collective_compute(
                    "AllGather",
                    ALU.bypass,
                    replica_groups=groups,
                    ins=[cc_in_k.opt()],
                    outs=[cc_out_k.opt()],
                )

            # -------- v shard + AllGather (pow2 payload) --------
            # v[jp, jtl, c] = sum_p x[jtl*128+jp, p]*Wv[p, c]; Wv read once.
            v_loc = locstage.tile([128, 4, H, D], BF16)
            psvs = [
                pp.tile([128, C], F32, tag=f"pp{jtl}", name=f"psum_v_{jtl}")
                for jtl in range(4)
            ]
            for q4 in range(PB // 4):
                wt4 = wstream.tile([128, 4, C], BF16, tag="wt", bufs=2,
                                   name="wt_v")
                nc.scalar.dma_start(
                    out=wt4,
                    in_=Wv[q4 * 512:(q4 + 1) * 512, :].rearrange(
                        "(q p) c -> p q c", p=128
                    ),
                )
                for q in range(4):
                    pb = q4 * 4 + q
                    for jtl in range(4):
                        nc.tensor.matmul(
                            psvs[jtl],
                            lhsT=xT[:, pb, jtl * 128:(jtl + 1) * 128],
                            rhs=wt4[:, q, :],
                            start=(pb == 0),
                            stop=(pb == PB - 1),
                        )
            for jtl in range(4):
                nc.vector.tensor_tensor(
                    out=v_loc[:, jtl, :, :],
                    in0=psvs[jtl].rearrange("p (h d) -> p h d", h=H),
                    in1=bv_b.rearrange("p (h d) -> p h d", h=H),
                    op=ALU.add,
                )
            nc.sync.dma_start(
                out=cc_in_v.rearrange("(jtl jp) f -> jp jtl f", jp=128),
                in_=v_loc,
            )
            if no_cc:
                for r in range(M):
                    nc.sync.dma_start(out=cc_out_v[r], in_=cc_in_v)
            else:
                nc.gpsimd.collective_compute(
                    "AllGather",
                    ALU.bypass,
                    replica_groups=groups,
                    ins=[cc_in_v.opt()],
                    outs=[cc_out_v.opt()],
                )

            # -------- q, skip, qe (overlap the AllGathers) --------
            projT(Wq, bq_sb, qT, scale=0.125)
            projT(Wskip, bskip_sb, outT)

            # qe8_b[:, h, i] = 0.125 * sum_d we[h*D+d]*qT[h*D+d, i]
            for h in range(H):
                cp0 = (h % 2) * D
                ct = h // 2
                pq = pp.tile([128, B], F32, tag="pq", bufs=2, name=f"psum_qe_{h}")
                nc.tensor.matmul(
                    pq,
                    lhsT=we_rep8[cp0:cp0 + D, h, :],
                    rhs=qT[cp0:cp0 + D, ct, :],
                )
                nc.vector.tensor_copy(out=qe8_b[:, h, :], in_=pq)

        if stage <= 1:
            for ct in range(CT):
                nc.sync.dma_start(
                    out=out[ct * 128:(ct + 1) * 128, :], in_=outT[:, ct, :]
                )
            return

        # ---------------- unpack gathered kT / v (contiguous HWDGE loads) ----
        for r in range(M):
            nc.sync.dma_start(
                out=kT[:, :, r * B:(r + 1) * B],
                in_=cc_out_k[r].rearrange("(ct cp) j -> cp ct j", cp=128),
            )
            for jtl in range(4):
                nc.scalar.dma_start(
                    out=v_aug[:, r * 4 + jtl, :, 0:D],
                    in_=cc_out_v[r, jtl * 128:(jtl + 1) * 128, :].rearrange(
                        "p (h d) -> p h d", h=H
                    ),
                )

        stats = small.tile([128, 2 * CT], F32, bufs=1)
        zall = keep.tile([H, B], F32)
        po_num = keep.tile([D, H, B], F32)

        # ---------------- attention ----------------
        # Grouped inner loop: qk matmuls for group g+1 are emitted before the
        # alpha@v matmuls of group g, so the PE queue streams back-to-back
        # matmuls (stays in its fast p-state) instead of gapping on the
        # vector->scalar->vector alpha chain. qT is pre-scaled by 0.125, so
        # the score add is a plain tensor add.
        GROUP = 4
        with (
            tc.tile_pool(name="psum_att", bufs=1, space="PSUM") as pa,
            tc.tile_pool(name="att", bufs=1) as att,
        ):
            for h in range(H):
                cp0 = (h % 2) * D
                ct = h // 2
                total_po = 2 * JT
                po = pa.tile([D + 1, B], F32, tag="po", bufs=2, name=f"po_{h}")
                po_k = 0

                def po_mm(lhsT, rhs):
                    nonlocal po_k
                    nc.tensor.matmul(
                        po,
                        lhsT=lhsT,
                        rhs=rhs,
                        start=(po_k == 0),
                        stop=(po_k == total_po - 1),
                        skip_group_check=True,
                    )
                    po_k += 1

                jt_groups = [
                    list(range(g, min(g + GROUP, JT)))
                    for g in range(0, JT, GROUP)
                ]
                ps_of = {}

                def emit_qk(jts):
                    for jt in jts:
                        ps = pa.tile([128, B], F32, tag="ps", bufs=5,
                                     name=f"ps_{h}_{jt}")
                        ps_of[jt] = ps
                        nc.tensor.matmul(
                            ps,
                            lhsT=kT[cp0:cp0 + D, ct, jt * 128:(jt + 1) * 128],
                            rhs=qT[cp0:cp0 + D, ct, :],
                            start=True,
                            stop=True,
                        )

                def emit_body(jts):
                    tiles = {}
                    for jt in jts:
                        tmp = att.tile([128, B], BF16, tag="tmp", bufs=6,
                                       name=f"tmp_{h}_{jt}")
                        nc.vector.tensor_tensor(
                            out=tmp, in0=xa[:, jt, :], in1=qe8_b[:, h, :],
                            op=ALU.mult,
                        )
                        ps2 = att.tile([128, B], BF16, tag="ps2", bufs=6,
                                       name=f"ps2_{h}_{jt}")
                        nc.vector.tensor_tensor(
                            out=ps2, in0=ps_of.pop(jt), in1=tmp, op=ALU.add
                        )
                        alpha = att.tile([128, B], BF16, tag="alpha", bufs=6,
                                         name=f"alpha_{h}_{jt}")
                        nc.scalar.activation(
                            out=alpha, in_=ps2, func=AF.Exp, scale=1.0
                        )
                        tiles[jt] = alpha
                    for jt in jts:
                        alpha = tiles[jt]
                        po_mm(v_aug[:, jt, h, :], alpha)
                        mt = att.tile([128, B], BF16, tag="mt", bufs=6,
                                      name=f"mt_{h}_{jt}")
                        nc.vector.tensor_tensor(
                            out=mt, in0=alpha, in1=xa[:, jt, :], op=ALU.mult
                        )
                        po_mm(we_aug_bf[:, h, :], mt)

                for g, jts in enumerate(jt_groups):
                    emit_qk(jts)
                    if g > 0:
                        emit_body(jt_groups[g - 1])
                emit_body(jt_groups[-1])
                assert po_k == total_po
                # epilogue part 1: stash numerator (SBUF) and Z row (zall[h];
                # partition h is DMA-reachable only)
                ztmp = small.tile([1, B], F32, tag="ztmp", name=f"ztmp_{h}")
                nc.vector.tensor_copy(out=ztmp, in_=po[D:D + 1, :])
                nc.sync.dma_start(out=zall[h:h + 1, :], in_=ztmp)
                nc.vector.tensor_copy(out=po_num[:, h, :], in_=po[0:D, :])

            # epilogue part 2: one batched reciprocal, then per-head
            # broadcast + multiply-accumulate into outT
            rzall = att.tile([H, B], F32)
            nc.vector.reciprocal(out=rzall, in_=zall)
            for h in range(H):
                cp0 = (h % 2) * D
                ct = h // 2
                prz = pa.tile([D, B], F32, tag="prz", bufs=1, name=f"prz2_{h}")
                nc.tensor.matmul(prz, lhsT=eye8x64[:, h, :], rhs=rzall)
                t1f = small.tile([128, B], F32, tag="t1", name=f"t1_{h}")
                t1 = t1f[cp0:cp0 + D, :]
                nc.vector.tensor_tensor(
                    out=t1, in0=po_num[:, h, :], in1=prz, op=ALU.mult
                )
                nc.vector.tensor_tensor(
                    out=outT[cp0:cp0 + D, ct, :],
                    in0=outT[cp0:cp0 + D, ct, :],
                    in1=t1,
                    op=ALU.add,
                )
                if h % 2 == 1:
                    # outT[:, ct, :] is final: fold its GraphNorm stats so the
                    # AllReduce fires right after the last head
                    sm = small.tile([128, 1], F32, tag="sm", name=f"sm_{ct}")
                    nc.vector.tensor_reduce(
                        out=sm, in_=outT[:, ct, :], axis=mybir.AxisListType.X,
                        op=ALU.add,
                    )
                    nc.vector.tensor_scalar(
                        out=stats[:, 2 * ct:2 * ct + 1], in0=sm,
                        scalar1=1.0 / N, scalar2=None, op0=ALU.mult,
                    )
                    scr = att.tile([128, B], F32, tag="scr", bufs=2,
                                   name=f"scr_{ct}")
                    nc.scalar.activation(
                        out=scr, in_=outT[:, ct, :], func=AF.Square
                    )
                    ss = small.tile([128, 1], F32, tag="ss", name=f"ss_{ct}")
                    nc.vector.tensor_reduce(
                        out=ss, in_=scr, axis=mybir.AxisListType.X, op=ALU.add
                    )
                    nc.vector.tensor_scalar(
                        out=stats[:, 2 * ct + 1:2 * ct + 2], in0=ss,
                        scalar1=1.0 / N, scalar2=None, op0=ALU.mult,
                    )

        if stage <= 2:
            for ct in range(CT):
                nc.sync.dma_start(
                    out=out[ct * 128:(ct + 1) * 128, :], in_=outT[:, ct, :]
                )
            return

        # ---------------- GraphNorm + L2 + emit ----------------
        with (
            tc.tile_pool(name="fin", bufs=1) as fin,
            tc.tile_pool(name="psum_f", bufs=1, space="PSUM") as pf,
        ):
            st_in = dram.tile([128, 2 * CT], F32)
            st_out = dram.tile([128, 2 * CT], F32, addr_space="Shared")
            nc.sync.dma_start(out=st_in, in_=stats)
            if no_cc:
                nc.sync.dma_start(out=st_out, in_=st_in)
            else:
                nc.gpsimd.collective_compute(
                    "AllReduce",
                    ALU.add,
                    replica_groups=groups,
                    ins=[st_in.opt()],
                    outs=[st_out.opt()],
                )

            # transpose raw outT while the AllReduce runs
            pre = fin.tile([128, 4, C], F32)  # [ip, it, c]
            for ct in range(CT):
                for it in range(4):
                    pt = pf.tile([128, 128], F32, tag="pt", bufs=4,
                                 name=f"pt_{ct}_{it}")
                    nc.tensor.transpose(
                        pt, outT[:, ct, it * 128:(it + 1) * 128], ident_f32
                    )
                    nc.vector.tensor_copy(
                        out=pre[:, it, ct * 128:(ct + 1) * 128], in_=pt
                    )

            gstats = small.tile([128, 2 * CT], F32, bufs=1)
            nc.sync.dma_start(out=gstats, in_=st_out)

            # per-channel A (scale) and Bc (shift), in one cvec tile so the
            # broadcast round-trip is a single DMA pair
            AB_cvec = small.tile([128, 2 * CT], F32, bufs=1)
            A_cvec = AB_cvec[:, 0:CT]
            B_cvec = AB_cvec[:, CT:2 * CT]
            for ct in range(CT):
                EX = gstats[:, 2 * ct:2 * ct + 1]
                EX2 = gstats[:, 2 * ct + 1:2 * ct + 2]
                msv = gnms_sb[:, ct:ct + 1]
                t2 = small.tile([128, 1], F32, tag="n_t", name=f"nt_{ct}")
                nc.vector.tensor_tensor(out=t2, in0=EX, in1=EX, op=ALU.mult)
                w1 = small.tile([128, 1], F32, tag="n_w", name=f"nw_{ct}")
                nc.vector.tensor_scalar(
                    out=w1, in0=msv, scalar1=-1.0, scalar2=2.0,
                    op0=ALU.mult, op1=ALU.add,
                )  # 2 - ms
                nc.vector.tensor_tensor(out=w1, in0=msv, in1=w1, op=ALU.mult)
                nc.vector.tensor_tensor(out=t2, in0=t2, in1=w1, op=ALU.mult)
                var = small.tile([128, 1], F32, tag="n_var", name=f"nvar_{ct}")
                nc.vector.tensor_tensor(out=var, in0=EX2, in1=t2, op=ALU.subtract)
                sd = small.tile([128, 1], F32, tag="n_sd", name=f"nsd_{ct}")
                nc.scalar.activation(out=sd, in_=var, func=AF.Sqrt, bias=eps_col)
                rstd = small.tile([128, 1], F32, tag="n_rstd", name=f"nrstd_{ct}")
                nc.vector.reciprocal(out=rstd, in_=sd)
                nc.vector.tensor_tensor(
                    out=A_cvec[:, ct:ct + 1], in0=gnw_sb[:, ct:ct + 1],
                    in1=rstd, op=ALU.mult,
                )
                p1 = small.tile([128, 1], F32, tag="n_p1", name=f"np1_{ct}")
                nc.vector.tensor_tensor(
                    out=p1, in0=A_cvec[:, ct:ct + 1], in1=msv, op=ALU.mult
                )
                nc.vector.tensor_tensor(out=p1, in0=p1, in1=EX, op=ALU.mult)
                nc.vector.tensor_tensor(
                    out=B_cvec[:, ct:ct + 1], in0=gnb_sb[:, ct:ct + 1],
                    in1=p1, op=ALU.subtract,
                )

            # broadcast A/Bc along partitions via one DRAM round trip (tiny)
            AB_dram = dram.tile([2 * C], F32)
            nc.gpsimd.dma_start(
                out=AB_dram.rearrange("(t p) -> p t", p=128), in_=AB_cvec
            )
            AB_bcast = fin.tile([128, 2 * C], F32)
            nc.gpsimd.dma_start(
                out=AB_bcast,
                in_=AB_dram.unsqueeze(0).partition_broadcast(128),
            )
            A_bcast = AB_bcast.rearrange("p (two c) -> p two c", two=2)[:, 0, :]
            B_bcast = AB_bcast.rearrange("p (two c) -> p two c", two=2)[:, 1, :]

            final = fin.tile([128, 4, C], F32)
            sqj = fin.tile([128, C], F32)
            for it in range(4):
                nc.vector.tensor_tensor(
                    out=final[:, it, :], in0=pre[:, it, :], in1=A_bcast,
                    op=ALU.mult,
                )
                nc.vector.tensor_tensor(
                    out=final[:, it, :], in0=final[:, it, :], in1=B_bcast,
                    op=ALU.add,
                )
                l2 = small.tile([128, 1], F32, tag="l2", name=f"l2_{it}")
                nc.scalar.activation(out=sqj, in_=final[:, it, :], func=AF.Square)
                nc.vector.tensor_reduce(
                    out=l2, in_=sqj, axis=mybir.AxisListType.X, op=ALU.add
                )
                sd2 = small.tile([128, 1], F32, tag="sd2", name=f"sd2_{it}")
                nc.scalar.activation(out=sd2, in_=l2, func=AF.Sqrt)
                rn = small.tile([128, 1], F32, tag="rn", name=f"rn_{it}")
                nc.vector.reciprocal(out=rn, in_=sd2)
                nc.vector.tensor_scalar(
                    out=final[:, it, :], in0=final[:, it, :],
                    scalar1=rn, scalar2=None, op0=ALU.mult,
                )
                nc.sync.dma_start(
                    out=out[it * 128:(it + 1) * 128, :], in_=final[:, it, :]
                )


def build_kernel(no_cc=False, n_cores=M, repeat=1, stage=3):
    nc = bacc.Bacc("TRN2", target_bir_lowering=False, debug=False,
                   num_devices=n_cores)

    xT_in = nc.dram_tensor("xT_bf", [N, B], BF16, kind="ExternalInput")
    xa_in = nc.dram_tensor("xa_bf", [N, B], BF16, kind="ExternalInput")
    Wq = nc.dram_tensor("Wq_bf", [N, C], BF16, kind="ExternalInput")
    Wk = nc.dram_tensor("Wk_bf", [N, C], BF16, kind="ExternalInput")
    Wv = nc.dram_tensor("Wv_bf", [N, C], BF16, kind="ExternalInput")
    Wskip = nc.dram_tensor("Wskip_bf", [N, C], BF16, kind="ExternalInput")
    bq = nc.dram_tensor("bq", [C], F32, kind="ExternalInput")
    bk = nc.dram_tensor("bk", [C], F32, kind="ExternalInput")
    bv = nc.dram_tensor("bv", [C], F32, kind="ExternalInput")
    bskip = nc.dram_tensor("bskip", [C], F32, kind="ExternalInput")
    we = nc.dram_tensor("we", [C], F32, kind="ExternalInput")
    gn_w = nc.dram_tensor("gn_w", [C], F32, kind="ExternalInput")
    gn_b = nc.dram_tensor("gn_b", [C], F32, kind="ExternalInput")
    gn_ms = nc.dram_tensor("gn_ms", [C], F32, kind="ExternalInput")
    out = nc.dram_tensor("out", [B, C], F32, kind="ExternalOutput")

    io = (xT_in, xa_in, Wq, Wk, Wv, Wskip, bq, bk, bv, bskip, we,
          gn_w, gn_b, gn_ms, out)
    groups = [list(range(n_cores))]

    with tile.TileContext(nc) as tc:
        for _rep in range(repeat):
            _emit_once(nc, tc, io, groups, no_cc, stage=stage)

    nc.finalize()
    return nc


_NC_CACHE = {}


def make_in_maps(inputs):
    """Host-side prep: slice per-core shards, pre-transpose x, cast to bf16."""
    x = np.ascontiguousarray(inputs["x"], dtype=np.float32)
    xT_bf = np.ascontiguousarray(x.T).astype(BF16_NP)   # [N, N]; col i = x[i,:]
    x_bf = x.astype(BF16_NP)
    w_bf = {
        k: np.ascontiguousarray(inputs[k], dtype=np.float32).astype(BF16_NP)
        for k in ("Wq", "Wk", "Wv", "Wskip")
    }
    f32v = {
        k: np.ascontiguousarray(inputs[k], dtype=np.float32)
        for k in ("bq", "bk", "bv", "bskip", "we", "gn_w", "gn_b", "gn_ms")
    }
    in_maps = []
    for m in range(M):
        I = slice(m * B, (m + 1) * B)
        im = {
            "xT_bf": np.ascontiguousarray(xT_bf[:, I]),
            "xa_bf": np.ascontiguousarray(x_bf[:, I]),
            "Wq_bf": w_bf["Wq"], "Wk_bf": w_bf["Wk"],
            "Wv_bf": w_bf["Wv"], "Wskip_bf": w_bf["Wskip"],
        }
        im.update(f32v)
        in_maps.append(im)
    return in_maps


def kernel(**inputs):
    if "nc" not in _NC_CACHE:
        _NC_CACHE["nc"] = build_kernel()
    nc = _NC_CACHE["nc"]
    in_maps = make_in_maps(inputs)
    res = run_bass_kernel_spmd(nc, in_maps, core_ids=list(range(M)))
    return np.concatenate([res.results[m]["out"] for m in range(M)], axis=0)


if __name__ == "__main__":
    data = np.load("/tmp/inputs.npz")
    out = kernel(**{k: data[k] for k in data.files})
    ref = np.load("/tmp/ref_out.npy")
    err = np.abs(out - ref)
    print("absmax", err.max(), "scale-rel", err.max() / np.abs(ref).max())
    print("rel2", np.linalg.norm(out - ref) / np.linalg.norm(ref))
